# revision 6
# baseline (speedup 1.0000x reference)
"""EMA (first-order linear recurrence) kernel for Trainium2, 8 NeuronCores.

Problem: y[b, t, d] = a*y[b, t-1, d] + (1-a)*x[b, t, d],  y[b, -1, d] = 0,
x shape (4, 4096, 2048) f32, ALPHA = 0.99.

Strategy (bf16 data path; the correctness gate is rel_err < 2e-2 and bf16
keeps us ~3e-3, while halving HBM traffic and tunnel transfer bytes):
  - Shard (batch, d_model/2) over the 8 cores: core (b, h) handles
    x[b, :, h*1024:(h+1)*1024] of shape (4096, 1024), cast to bf16 on host.
  - Chunked scan over seq: 32 blocks of L=128 steps, grouped by 16 for the
    carry computation.  For each block j:
      w-matmul: stationary one-hot-column matrix (col j = (1-a)*a^(127-k))
        accumulates the block's local end-state w_j into row j of a
        persistent PSUM tile W_psum [32, 1024] (+0 rows elsewhere).
    Per group of 16 blocks (after their w-matmuls):
      ScalarE/VectorE copy the known prefix W_psum -> SBUF (bf16);
      one small matmul computes all carries SC*e_j (SC = a/(1-a));
      carries are staged to Ca/Cb [16, DS] bf16.
    Main matmul pair per (block, half): Y_j = T @ X_j + Psel_q.T @ Ch
      where Psel_q [16, 128] has row q = T[:, 0] (so the accumulated term
      is a^(i+1) * e_j, exactly the carry contribution), both into the same
      PSUM accumulation group; one engine copy (with f32->bf16 cast) to an
      output staging tile, one 1-MiB store DMA per 4 blocks.
    No SWDGE (gpsimd) DMAs anywhere: the current walrus build miscompiles
    the For_i loop-reset InstIncSwdgeSem ("ISA wrong length"), so the whole
    kernel uses HWDGE (sync/scalar) DMAs only.
  - X is resident in SBUF (64 KiB/partition bf16); loads are 8 DMAs of
    1 MiB (4 blocks each, rearranged so partition = t-within-block).

Dispatch: the jitted shard_map executable, the device-resident zero output
buffers, and the Bass module are all cached at module level, so a warm
kernel() call does no re-tracing and ships only x (bf16) through the axon
tunnel and y (bf16) back.

The walrus build in this container supports only ONE sync-wait per
instruction; _legalize_waits splits extras onto same-engine NOPs (see
baseline notes), which is semantics-preserving because engines execute
their streams in order.
"""
import numpy as np
from contextlib import ExitStack

ALPHA = 0.99
B, S, DM = 4, 4096, 2048
NCORES = 8
DS = 1024                  # d-columns per core (d_model/2)
L = 128                    # seq block length
NB = S // L                # 32 blocks
HF = 16                    # blocks per carry half (two carry chains total)
HW2 = 32                   # rows of each half's w-PSUM tile (matmul M)
NH = 512                   # matmul moving-operand half (PSUM f32 bank limit)
LCH = 8                    # blocks per load DMA (1 MiB fp8), sync/SP ring
SCH = 4                    # blocks per store DMA (1 MiB bf16), scalar/ACT ring
FP8_IN = True              # sigma-delta fp8-e3m4 input (halves load bytes)

_cache = {}


# ---------------------------------------------------------------------------
# walrus wait-count legalization
# ---------------------------------------------------------------------------
def _legalize_waits(nc, max_waits=1, matmul_max=0):
    import concourse.mybir as mybir

    counter = [0]

    def split(blk):
        insts = blk.instructions
        i = 0
        while i < len(insts):
            inst = insts[i]
            for sub in (getattr(inst, "blocks", None) or []):
                split(sub)
            si = inst.sync_info
            cap = matmul_max if isinstance(inst, mybir.InstMatmult) else max_waits
            if si is not None and si.on_wait and len(si.on_wait) > cap:
                waits = list(si.on_wait)
                keep = waits[len(waits) - cap:] if cap > 0 else []
                overflow = waits[: len(waits) - cap]
                nops = []
                for j in range(0, len(overflow), max_waits):
                    chunk = overflow[j: j + max_waits]
                    counter[0] += 1
                    nop = mybir.InstNoOp(name=f"wsplit_nop_{counter[0]}")
                    nop.engine = inst.engine
                    nop.sync_info = mybir.SyncInfo(on_wait=chunk, on_update=[])
                    nops.append(nop)
                inst.sync_info = mybir.SyncInfo(
                    on_wait=keep, on_update=list(si.on_update)
                )
                for k, nop in enumerate(nops):
                    insts.insert(i + k, nop)
                i += len(nops)
            i += 1

    for fn in nc.m.functions:
        for blk in fn.blocks:
            split(blk)
    return nc


# ---------------------------------------------------------------------------
# constants
# ---------------------------------------------------------------------------
def _np_bf16():
    import ml_dtypes

    return np.dtype(ml_dtypes.bfloat16)


def _constants():
    a = float(ALPHA)
    bf16 = _np_bf16()
    ii = np.arange(L)
    diff = ii[None, :] - ii[:, None]              # i - k
    # tT[k, i] = T[i, k] = (1-a) * a^(i-k) for k <= i else 0
    tT = np.where(
        diff >= 0,
        (1.0 - a) * np.power(a, np.clip(diff, 0, None).astype(np.float64)),
        0.0,
    ).astype(bf16)
    # wsel[:, q*HW2:(q+1)*HW2] is the stationary [L, HW2] for the w-matmul of
    # block j with q = j % HF: only column q is nonzero, = (1-a) * a^(L-1-k).
    tlast = (1.0 - a) * np.power(a, (L - 1 - ii).astype(np.float64))
    wsel = np.zeros((L, HF * HW2), dtype=np.float64)
    for q in range(HF):
        wsel[:, q * HW2 + q] = tlast
    wsel = wsel.astype(bf16)
    # Carry-matrix columns produce the row-0 correction SC*e_j directly
    # (SC = a/(1-a)).  Chain 0 (blocks 0..HF-1): sT16[m, q] over w_0..w_q-1.
    aL = a ** L
    SC = a / (1.0 - a)
    q = np.arange(HF)
    r = np.arange(HF)
    d0 = q[None, :] - 1 - r[:, None]
    sT16 = np.where(
        d0 >= 0, SC * np.power(aL, np.clip(d0, 0, None).astype(np.float64)), 0.0
    ).astype(bf16)
    # Chain 1 (blocks HF..NB-1): operates on Ws[0:48] where rows 0-15 hold
    # w_0..15, rows 16-31 are zero padding, rows 32-47 hold w_16..31.
    sT48 = np.zeros((3 * HF, HF))
    sT48[0:HF] = SC * np.power(
        aL, (HF + q[None, :] - 1 - r[:, None]).astype(np.float64)
    )
    sT48[2 * HF:3 * HF] = np.where(
        d0 >= 0, SC * np.power(aL, np.clip(d0, 0, None).astype(np.float64)), 0.0
    )
    sT48 = sT48.astype(bf16)
    # Fixup stationary: Psel[q', q*L + i] = (1-a)*a^i for q'==q else 0.
    # matmul(Psel_q, Ch) adds (1-a)a^i * SC*e_j = a^(i+1)*e_j to Y_j, which
    # is exactly the global-carry contribution for block j.
    pcol = (1.0 - a) * np.power(a, ii.astype(np.float64))
    psel = np.zeros((HF, HF * L), dtype=np.float64)
    for qq in range(HF):
        psel[qq, qq * L:(qq + 1) * L] = pcol
    psel = psel.astype(bf16)
    return tT, wsel, sT16, sT48, psel


def _build_nc(reps=1):
    import concourse.bass as bass
    import concourse.tile as tile
    from concourse import mybir

    f32 = mybir.dt.float32
    bf16 = mybir.dt.bfloat16
    xdt = mybir.dt.float8e3 if FP8_IN else bf16
    tT_np, wsel_np, sT16_np, sT48_np, psel_np = _constants()

    nc = bass.Bass("TRN2", target_bir_lowering=False, debug=False)
    x = nc.dram_tensor("x_sh", [S, DS], xdt, kind="ExternalInput")
    y = nc.dram_tensor("y_sh", [S, DS], bf16, kind="ExternalOutput")
    tT_d = nc.inline_tensor(tT_np, name="tT_const")
    wsel_d = nc.inline_tensor(wsel_np, name="wsel_const")
    sT16_d = nc.inline_tensor(sT16_np, name="sT16_const")
    sT48_d = nc.inline_tensor(sT48_np, name="sT48_const")
    psel_d = nc.inline_tensor(psel_np, name="psel_const")

    with ExitStack() as ctx:
        tc = ctx.enter_context(tile.TileContext(nc))
        cpool = ctx.enter_context(tc.tile_pool(name="cpool", bufs=1))
        xpool = ctx.enter_context(tc.tile_pool(name="xpool", bufs=2))
        opool = ctx.enter_context(tc.tile_pool(name="opool", bufs=3))
        pwAB = ctx.enter_context(tc.tile_pool(name="pwAB", bufs=1, space="PSUM"))
        pp = ctx.enter_context(tc.tile_pool(name="pp", bufs=4, space="PSUM"))

        tT = cpool.tile([L, L], bf16)
        nc.sync.dma_start(tT[:], tT_d.ap())
        wsel = cpool.tile([L, HF * HW2], bf16)
        nc.sync.dma_start(wsel[:], wsel_d.ap())
        sT16 = cpool.tile([HF, HF], bf16)
        nc.sync.dma_start(sT16[:], sT16_d.ap())
        sT48 = cpool.tile([3 * HF, HF], bf16)
        nc.sync.dma_start(sT48[:], sT48_d.ap())
        psel = cpool.tile([HF, HF * L], bf16)
        nc.sync.dma_start(psel[:], psel_d.ap())

        Ws = cpool.tile([2 * HW2, DS], bf16, tag="Ws")  # [64, DS] end states
        Ca = cpool.tile([HF, DS], bf16, tag="Ca")       # SC*e_j half 0
        Cb = cpool.tile([HF, DS], bf16, tag="Cb")       # SC*e_j half 1

        xap, yap = x.ap(), y.ap()

        rep_loop = tc.For_i(0, reps, 1) if reps > 1 else None
        if rep_loop is not None:
            rep_loop.__enter__()
        if True:
            # X double-buffers across rep iterations so the next rep's load
            # DMAs overlap this rep's compute + stores (xpool bufs=2).
            X = xpool.tile([L, NB * DS], xdt, tag="X", name="X")
            # ---------------- loads: 4 x 1 MiB fp8, in block order ------
            for qd in range(NB // LCH):
                dst = X[:, qd * LCH * DS:(qd + 1) * LCH * DS].rearrange(
                    "p (b n) -> p b n", b=LCH
                )
                src = xap[qd * LCH * L:(qd + 1) * LCH * L, :].rearrange(
                    "(b p) n -> p b n", b=LCH
                )
                nc.sync.dma_start(dst, src)

            def issue_w(hb, wp):
                j0 = hb * HF
                for j in range(j0, j0 + HF):
                    qsel = j - j0
                    for h in range(DS // NH):
                        nc.tensor.matmul(
                            wp[:, h * NH:(h + 1) * NH],
                            wsel[:, qsel * HW2:(qsel + 1) * HW2],
                            X[:, j * DS + h * NH: j * DS + (h + 1) * NH],
                            start=(qsel == 0), stop=(qsel == HF - 1),
                        )

            def issue_chain(hb, wp):
                # prefix copies split by free halves across the two engines;
                # half hb's w rows land at Ws[hb*32 : hb*32+32] (32-aligned)
                w0 = hb * HW2
                nc.scalar.copy(Ws[w0:w0 + HW2, 0:NH], wp[:, 0:NH])
                nc.vector.tensor_copy(Ws[w0:w0 + HW2, NH:DS], wp[:, NH:DS])
                sTh = sT16 if hb == 0 else sT48
                kh = HF if hb == 0 else 3 * HF
                Ch = Ca if hb == 0 else Cb
                for h in range(DS // NH):
                    ct = pp.tile([L, NH], f32, tag="pp", name=f"ct{hb}_{h}")
                    nc.tensor.matmul(
                        ct[0:HF, :],
                        sTh[:],
                        Ws[0:kh, h * NH:(h + 1) * NH],
                        start=True, stop=True,
                    )
                    if h == 0:
                        nc.scalar.copy(Ch[:, 0:NH], ct[0:HF, :])
                    else:
                        nc.vector.tensor_copy(Ch[:, NH:DS], ct[0:HF, :])

            def issue_mains(hb):
                j0 = hb * HF
                Ch = Ca if hb == 0 else Cb
                ot = None
                for j in range(j0, j0 + HF):
                    qsel = j - j0
                    if j % SCH == 0:
                        ot = opool.tile(
                            [L, SCH * DS], bf16, tag="ot", name=f"ot{j // SCH}"
                        )
                    for h in range(DS // NH):
                        pt = pp.tile([L, NH], f32, tag="pp", name=f"pt{j}_{h}")
                        nc.tensor.matmul(
                            pt[:],
                            tT[:],
                            X[:, j * DS + h * NH: j * DS + (h + 1) * NH],
                            start=True, stop=False,
                        )
                        nc.tensor.matmul(
                            pt[:],
                            psel[:, qsel * L:(qsel + 1) * L],
                            Ch[:, h * NH:(h + 1) * NH],
                            start=False, stop=True,
                        )
                        dst_sl = ot[
                            :,
                            (j % SCH) * DS + h * NH:(j % SCH) * DS + (h + 1) * NH,
                        ]
                        if (2 * j + h) % 2 == 0:
                            nc.scalar.copy(dst_sl, pt[:])
                        else:
                            nc.vector.tensor_copy(dst_sl, pt[:])
                    if j % SCH == SCH - 1:
                        jb = j - (SCH - 1)
                        nc.scalar.dma_start(
                            yap[jb * L:(j + 1) * L, :].rearrange(
                                "(b p) n -> p b n", b=SCH
                            ),
                            ot[:].rearrange("p (b n) -> p b n", b=SCH),
                        )

            # software pipeline: half 1's w-matmuls are independent of half
            # 0's chain/mains, so they fill the PE while chain 0 resolves.
            A = pwAB.tile([HW2, DS], f32, tag="A", name="Aw")
            Bp = pwAB.tile([HW2, DS], f32, tag="B", name="Bw")
            issue_w(0, A)
            issue_chain(0, A)
            issue_w(1, Bp)
            issue_mains(0)
            issue_chain(1, Bp)
            issue_mains(1)
        if rep_loop is not None:
            rep_loop.__exit__(None, None, None)
    return _legalize_waits(nc)


def _get_nc():
    if "nc" not in _cache:
        _cache["nc"] = _build_nc()
    return _cache["nc"]


# ---------------------------------------------------------------------------
# cached jitted dispatch (mirrors bass2jax.run_bass_via_pjrt, but the traced
# executable, mesh, and zero output buffers are built once and reused)
# ---------------------------------------------------------------------------
def _get_exec():
    if "exec" in _cache:
        return _cache["exec"]
    import jax
    from jax.sharding import Mesh, PartitionSpec, NamedSharding
    from jax.experimental.shard_map import shard_map
    import concourse.mybir as mybir
    from concourse import bass2jax

    bass2jax.install_neuronx_cc_hook()
    nc = _get_nc()

    partition_name = nc.partition_id_tensor.name if nc.partition_id_tensor else None
    in_names, out_names, out_avals = [], [], []
    for alloc in nc.m.functions[0].allocations:
        if not isinstance(alloc, mybir.MemoryLocationSet):
            continue
        name = alloc.memorylocations[0].name
        if alloc.kind == "ExternalInput":
            if name != partition_name:
                in_names.append(name)
        elif alloc.kind == "ExternalOutput":
            out_names.append(name)
            out_avals.append(
                jax.core.ShapedArray(
                    tuple(alloc.tensor_shape), mybir.dt.np(alloc.dtype)
                )
            )
    all_names = list(in_names) + list(out_names)
    if partition_name is not None:
        all_names.append(partition_name)

    def _body(*args):
        operands = list(args)
        if partition_name is not None:
            operands.append(bass2jax.partition_id_tensor())
        return tuple(
            bass2jax._bass_exec_p.bind(
                *operands,
                out_avals=tuple(out_avals),
                in_names=tuple(all_names),
                out_names=tuple(out_names),
                lowering_input_output_aliases=(),
                sim_require_finite=True,
                sim_require_nnan=True,
                nc=nc,
            )
        )

    devices = jax.devices()[:NCORES]
    mesh = Mesh(np.asarray(devices), ("core",))
    nin = len(in_names) + len(out_names)
    sharded = jax.jit(
        shard_map(
            _body, mesh=mesh,
            in_specs=(PartitionSpec("core"),) * nin,
            out_specs=(PartitionSpec("core"),) * len(out_names),
            check_rep=False,
        ),
        keep_unused=True,
    )
    sharding = NamedSharding(mesh, PartitionSpec("core"))
    # device-resident zero buffers for the ExternalOutput operands; NOT
    # donated, so they are created once and reused every call.
    zeros = [
        jax.device_put(
            np.zeros((NCORES * a.shape[0], *a.shape[1:]), a.dtype), sharding
        )
        for a in out_avals
    ]
    _cache["exec"] = (sharded, sharding, devices, zeros)
    return _cache["exec"]


def _sigma_delta_fp8(x):
    """First-order noise-shaped fp8-e3m4 quantization along t.

    Error feedback high-passes the quantization error; the EMA low-pass then
    attenuates it by ~(1-a)*sqrt(2/(1+a)) ~ 0.01, so the device-side scan on
    the fp8 stream stays within ~3e-3 of the f32 reference.
    """
    import ml_dtypes

    f8 = np.dtype(ml_dtypes.float8_e3m4)
    q = np.empty(x.shape, dtype=f8)
    e = np.zeros((x.shape[0], x.shape[2]), dtype=np.float32)
    for t in range(x.shape[1]):
        v = x[:, t, :] + e
        qt = v.astype(f8)
        q[:, t, :] = qt
        e = v - qt.astype(np.float32)
    return q


def _shard_cast(x):
    """Per-core input slices: core c = (b, h) -> x[b, :, h*DS:(h+1)*DS]."""
    xq = _sigma_delta_fp8(x) if FP8_IN else x.astype(_np_bf16())
    return [
        xq[c // 2, :, (c % 2) * DS:((c % 2) + 1) * DS]
        for c in range(NCORES)
    ]


def kernel(x) -> np.ndarray:
    import jax

    x = np.asarray(x, dtype=np.float32)
    assert x.shape == (B, S, DM), x.shape
    sharded, sharding, devices, zeros = _get_exec()
    parts = _shard_cast(x)
    dparts = [jax.device_put(parts[c], devices[c]) for c in range(NCORES)]
    xg = jax.make_array_from_single_device_arrays(
        (NCORES * S, DS), sharding, dparts
    )
    outs = sharded(xg, *zeros)
    g = np.asarray(outs[0]).reshape(B, 2, S, DS)
    out = np.empty((B, S, DM), dtype=np.float32)
    out[:, :, :DS] = g[:, 0]
    out[:, :, DS:] = g[:, 1]
    return out
